# revision 22
# baseline (speedup 1.0000x reference)
"""Trainium2 Bass kernel for nn_Decoder_85916525789418 (GRU decoder with
per-scene self-attention), data-parallel over scenes across 8 NeuronCores.

Contract: kernel(**inputs) takes the FULL unsharded inputs (as produced by
reference.setup_inputs) and returns the full (mus, stds) outputs.

Design notes (on-device layout is "T layout": feature dims on SBUF
partitions, pedestrians along the free axis):
  - batch axis (65536 peds = 4096 scenes x 16) sharded 8 ways at scene
    granularity -> 8192 peds/core, no cross-core communication.
  - host does layout prep only (transpose/slice/pack); all FLOPs on device.
  - per timestep: PSUM-accumulated GRU gates (W_hh @ h + W_ia @ a), DVE/ACT
    elementwise GRU, per-128-ped-group attention via symmetric score matmuls,
    masked-exp softmax (TTR for denominators), PE transposes for h_row/attnT,
    fc_attn, and a cheap [128,4]-wide mu/std matmul DMA'd straight out.
"""

import sys

for _p in ("/opt/trn_rl_repo",):
    if _p not in sys.path:
        sys.path.insert(0, _p)

import numpy as np
import ml_dtypes

import concourse.bass as bass
import concourse.mybir as mybir
from concourse import bacc, tile
from concourse.bass import ts, ds

NCORES = 8
B, PED, H, MLP, ZD, TT, NS, NP = 65536, 16, 128, 256, 32, 12, 6, 2
ZX = MLP + ZD          # 288
G3 = 3 * H             # 384
BL = B // NCORES       # 8192 peds per core
NB = 512               # peds per processing tile
NGRP = NB // 128       # 4 groups of 128 peds per tile

F32 = mybir.dt.float32
F32R = mybir.dt.float32r
BF16 = mybir.dt.bfloat16
FP16 = mybir.dt.float16
AF = mybir.ActivationFunctionType
OP = mybir.AluOpType
BF16NP = ml_dtypes.bfloat16
FP16NP = np.float16


def build_module(bl=BL, t_steps=TT, stage=99):
    """Build the Bass module for one core processing `bl` peds."""
    nt = bl // NB
    nc = bacc.Bacc("TRN2", target_bir_lowering=False)

    # ---- DRAM I/O ----
    d_zxT = nc.dram_tensor("zxT", [ZX, bl], F32R, kind="ExternalInput")
    d_lsT = nc.dram_tensor("lsT", [NS, bl], F32R, kind="ExternalInput")
    d_futT = nc.dram_tensor("futT", [t_steps * NP, bl], FP16, kind="ExternalInput")
    d_whhT = nc.dram_tensor("whhT", [H, G3], FP16, kind="ExternalInput")
    d_wiaT = nc.dram_tensor("wiaT", [NP, G3], FP16, kind="ExternalInput")
    d_wihxT = nc.dram_tensor("wihxT", [ZX, G3], F32R, kind="ExternalInput")
    d_wdecT = nc.dram_tensor("wdecT", [ZX, H], F32R, kind="ExternalInput")
    d_wvelT = nc.dram_tensor("wvelT", [NS, NP], F32R, kind="ExternalInput")
    d_wattnT = nc.dram_tensor("wattnT", [2 * H, H], FP16, kind="ExternalInput")
    d_wms = nc.dram_tensor("wms", [H, 2 * NP], FP16, kind="ExternalInput")
    d_bms = nc.dram_tensor("bms", [1, 2 * NP], FP16, kind="ExternalInput")
    d_bvel = nc.dram_tensor("bvel", [1, NP], F32R, kind="ExternalInput")
    d_bhh05 = nc.dram_tensor("bhh05", [1, 128], F32R, kind="ExternalInput")
    # bias_gx[:, c] = b_ih[c-chunk] (+ b_hh[c-chunk] for r,z chunks only)
    d_biasgx = nc.dram_tensor("biasgx", [H, 3], F32, kind="ExternalInput")
    d_bhhn = nc.dram_tensor("bhhn", [H, 1], F32, kind="ExternalInput")
    d_bhhnh = nc.dram_tensor("bhhnh", [H, 1], F32, kind="ExternalInput")
    d_battn = nc.dram_tensor("battn", [H, 1], F32, kind="ExternalInput")
    d_bdec = nc.dram_tensor("bdec", [H, 1], F32, kind="ExternalInput")
    d_ident = nc.dram_tensor("ident", [128, 128], FP16, kind="ExternalInput")
    d_blkmask = nc.dram_tensor("blkmask", [128, NB], BF16, kind="ExternalInput")
    d_wat2f = nc.dram_tensor("wat2f", [H, H], F32R, kind="ExternalInput")
    d_onesb = nc.dram_tensor("onesb", [1, NB], FP16, kind="ExternalInput")
    d_onesf = nc.dram_tensor("onesf", [1, NB], F32R, kind="ExternalInput")
    # out[t, g, c, p]: c in (mu0, mu1, std0, std1), g = global group id, p = ped in group
    d_out = nc.dram_tensor("outT", [t_steps, bl // 128, 4, 128], F32,
                           kind="ExternalOutput")

    with tile.TileContext(nc) as tc:
        with (
            tc.tile_pool(name="singles", bufs=1) as singles,
            tc.tile_pool(name="zxp", bufs=2) as zxp,
            tc.tile_pool(name="gwork", bufs=4) as gwork,
            tc.tile_pool(name="awork", bufs=4) as awork,
            tc.tile_pool(name="psum", bufs=1, space="PSUM") as psum,
        ):
            # ---- persistent SBUF state ----
            hT = singles.tile([128, bl], FP16)       # hidden state, T layout
            gx0 = singles.tile([128, bl], FP16)      # gx_zx r chunk (+biases)
            gx1 = singles.tile([128, bl], FP16)      # z chunk
            gx2 = singles.tile([128, bl], FP16)      # n chunk (b_ih only)
            a_bufs = [singles.tile([NP, bl], FP16, name=f"acur{j}") for j in range(2)]

            whhT = singles.tile([H, G3], FP16)
            wiaT = singles.tile([NP, G3], FP16)
            wihx0 = singles.tile([128, G3], F32R)
            wihx1 = singles.tile([128, G3], F32R)
            wihx2 = singles.tile([ZX - 256, G3], F32R)
            wdec0 = singles.tile([128, H], F32R)
            wdec1 = singles.tile([128, H], F32R)
            wdec2 = singles.tile([ZX - 256, H], F32R)
            wvelT = singles.tile([NS, NP], F32R)
            wat1 = singles.tile([H, H], FP16)
            wat2 = singles.tile([H, H], FP16)
            wms = singles.tile([H, 2 * NP], FP16)
            bms = singles.tile([1, 2 * NP], FP16)
            bvel = singles.tile([1, NP], F32R)
            bhh05 = singles.tile([1, 128], F32R)
            biasgx = singles.tile([H, 3], F32)
            bhhn = singles.tile([H, 1], F32)
            bhhnh = singles.tile([H, 1], F32)
            battn = singles.tile([H, 1], F32)
            bdec = singles.tile([H, 1], F32)
            ident = singles.tile([128, 128], FP16)
            blkmask = singles.tile([128, NB], BF16)
            wat2f = singles.tile([H, H], F32R)
            msbufs = [singles.tile([128, 16 * (bl // NB)], F32, name=f"msb{j}") for j in range(2)]
            onesb = singles.tile([1, NB], FP16)
            onesf = singles.tile([1, NB], F32R)

            for dst, src in [
                (whhT, d_whhT), (wiaT, d_wiaT),
                (wvelT, d_wvelT), (wms, d_wms), (bms, d_bms), (bvel, d_bvel),
                (bhh05, d_bhh05),
                (biasgx, d_biasgx), (bhhn, d_bhhn), (bhhnh, d_bhhnh), (battn, d_battn),
                (bdec, d_bdec), (ident, d_ident), (blkmask, d_blkmask),
                (onesb, d_onesb), (onesf, d_onesf), (wat2f, d_wat2f),
            ]:
                nc.sync.dma_start(dst[:], src[:])
            nc.sync.dma_start(wihx0[:], d_wihxT[0:128, :])
            nc.sync.dma_start(wihx1[:], d_wihxT[128:256, :])
            nc.sync.dma_start(wihx2[:], d_wihxT[256:ZX, :])
            nc.sync.dma_start(wdec0[:], d_wdecT[0:128, :])
            nc.sync.dma_start(wdec1[:], d_wdecT[128:256, :])
            nc.sync.dma_start(wdec2[:], d_wdecT[256:ZX, :])
            nc.sync.dma_start(wat1[:], d_wattnT[0:128, :])
            nc.sync.dma_start(wat2[:], d_wattnT[128:256, :])

            # ---- pre-loop: gx_zx, h0, a0 ----
            for i in range(nt):
                sl = ts(i, NB)
                z0 = zxp.tile([128, NB], F32R, tag="z0")
                z1 = zxp.tile([128, NB], F32R, tag="z1")
                z2 = zxp.tile([ZX - 256, NB], F32R, tag="z2")
                nc.sync.dma_start(z0[:], d_zxT[0:128, sl])
                nc.sync.dma_start(z1[:], d_zxT[128:256, sl])
                nc.sync.dma_start(z2[:], d_zxT[256:ZX, sl])
                for oc, gxs in enumerate((gx0, gx1, gx2)):
                    ps = psum.tile([128, NB], F32, tag=["pra", "prz", "prn"][oc])
                    nc.tensor.matmul(ps[:], wihx0[:, ts(oc, 128)],
                                     z0[:], start=True, stop=False)
                    nc.tensor.matmul(ps[:], wihx1[:, ts(oc, 128)],
                                     z1[:], start=False, stop=False)
                    nc.tensor.matmul(ps[:], wihx2[:, ts(oc, 128)],
                                     z2[:], start=False, stop=True)
                    if oc == 1:
                        nc.scalar.activation(gxs[:, sl], ps[:], AF.Identity,
                                             bias=biasgx[:, oc:oc + 1])
                    else:
                        nc.vector.tensor_scalar(
                            gxs[:, sl], ps[:], biasgx[:, oc:oc + 1], None,
                            OP.add)
                # h0
                ps = psum.tile([128, NB], F32, tag="prn2")
                nc.tensor.matmul(ps[:], wdec0[:],
                                 z0[:], start=True, stop=False)
                nc.tensor.matmul(ps[:], wdec1[:],
                                 z1[:], start=False, stop=False)
                nc.tensor.matmul(ps[:], wdec2[:],
                                 z2[:], start=False, stop=True)
                nc.scalar.activation(hT[:, sl], ps[:], AF.Identity,
                                     bias=bdec[:, 0:1])
                # a0 = last_state @ W_vel.T + b_vel  -> aT rows 0:2
                lst = zxp.tile([NS, NB], F32R, tag="ls")
                nc.sync.dma_start(lst[:], d_lsT[:, sl])
                psa = psum.tile([NP, NB], F32, tag="psc")
                nc.tensor.matmul(psa[:], wvelT[:],
                                 lst[:], start=True, stop=False)
                nc.tensor.matmul(psa[:], bvel[:],
                                 onesf[:], start=False, stop=True)
                nc.scalar.copy(a_bufs[0][0:NP, sl], psa[:])

            # ---- time loop ----
            for t in range(t_steps):
                a_cur = a_bufs[t % 2]
                msbuf = msbufs[t % 2]
                if t > 0:
                    # a_t = fut_state[t-1]
                    nc.sync.dma_start(a_cur[:], d_futT[ds(NP * (t - 1), NP), :])
                # --- phase A: GRU gates + state update (all tiles) ---
                for i in range(nt):
                    sl = ts(i, NB)
                    psr = psum.tile([128, NB], F32, tag="pra")
                    psz = psum.tile([128, NB], F32, tag="prz")
                    psn = psum.tile([128, NB], F32, tag="prn")
                    psn2 = psum.tile([128, NB], F32, tag="prn2")
                    nc.tensor.matmul(psr[:], whhT[:, 0:128], hT[:, sl],
                                     start=True, stop=False)
                    nc.tensor.matmul(psr[:], wiaT[:, 0:128], a_cur[:, sl],
                                     start=False, stop=False)
                    nc.tensor.matmul(psr[:], ident[:], gx0[:, sl],
                                     start=False, stop=True)
                    nc.tensor.matmul(psz[:], whhT[:, 128:256], hT[:, sl],
                                     start=True, stop=False)
                    nc.tensor.matmul(psz[:], wiaT[:, 128:256], a_cur[:, sl],
                                     start=False, stop=False)
                    nc.tensor.matmul(psz[:], ident[:], gx1[:, sl],
                                     start=False, stop=True)
                    nc.tensor.matmul(psn[:], whhT[:, 256:384], hT[:, sl],
                                     start=True, stop=False)
                    nc.tensor.matmul(psn[:], bhh05[:], onesf[:],
                                     start=False, stop=True)
                    nc.tensor.matmul(psn2[:], wiaT[:, 256:384], a_cur[:, sl],
                                     start=True, stop=False)
                    nc.tensor.matmul(psn2[:], ident[:], gx2[:, sl],
                                     start=False, stop=True)

                    if stage < 2:
                        continue
                    wr = gwork.tile([128, NB], FP16, tag="wr", bufs=6)
                    wz = gwork.tile([128, NB], FP16, tag="wz", bufs=6)
                    zg = gwork.tile([128, NB], FP16, tag="zg")
                    tmp = gwork.tile([128, NB], FP16, tag="tmp", bufs=6)
                    npre = gwork.tile([128, NB], FP16, tag="npre", bufs=6)
                    ng = gwork.tile([128, NB], FP16, tag="ng", bufs=6)
                    dd = gwork.tile([128, NB], FP16, tag="dd")
                    m2 = gwork.tile([128, NB], FP16, tag="m2")

                    # sigmoid(x) = 0.5*tanh(x/2) + 0.5 -> single ACT table set
                    nc.scalar.activation(wr[:], psr[:], AF.Tanh, scale=0.5)
                    nc.scalar.activation(wz[:], psz[:], AF.Tanh, scale=0.5)
                    # tmp = (wr+1)*psn = r*(gh_n+b_hh_n)
                    nc.vector.scalar_tensor_tensor(
                        tmp[:], wr[:], 1.0, psn[:], OP.add, OP.mult)
                    # npre = (ga_n + gx_n) + tmp
                    nc.vector.scalar_tensor_tensor(
                        npre[:], psn2[:], 1.0, tmp[:], OP.mult, OP.add)
                    nc.scalar.activation(ng[:], npre[:], AF.Tanh)
                    # h~ = ng + z*(h - ng), z = 0.5*wz + 0.5
                    nc.gpsimd.tensor_scalar(zg[:], wz[:], 0.5, 0.5,
                                            OP.mult, OP.add)
                    nc.gpsimd.tensor_sub(dd[:], hT[:, sl], ng[:])
                    nc.vector.tensor_mul(m2[:], zg[:], dd[:])
                    nc.vector.tensor_add(hT[:, sl], ng[:], m2[:])

                # --- phase B: attention + fc + outputs (all tiles) ---
                if stage < 3:
                    continue
                for i in range(nt):
                    sl = ts(i, NB)
                    pssc = psum.tile([128, NB], F32, tag="psc")
                    pstr = psum.tile([128, 2 * NB], FP16, tag="ptr")
                    psth = pstr[:, 0:NB]
                    psta = pstr[:, NB:2 * NB]
                    psctx = psum.tile([128, NB], F32, tag="pctx")
                    expv = awork.tile([128, NB], BF16, tag="expv", bufs=5)
                    mexp = awork.tile([128, NB], BF16, tag="mexp", bufs=5)
                    attn = awork.tile([128, NB], FP16, tag="attn", bufs=5)
                    den = awork.tile([128, NGRP], F32, tag="den")
                    rden = awork.tile([128, NGRP], F32, tag="rden")
                    hrow = awork.tile([128, NB], FP16, tag="hrow", bufs=5)
                    attnT = awork.tile([128, NB], FP16, tag="attnT", bufs=5)
                    ctxT = awork.tile([128, NB], FP16, tag="ctxT", bufs=5)

                    for g in range(NGRP):
                        go = ts(g, 128)
                        ab = ds(i * NB + g * 128, 128)
                        nc.tensor.matmul(pssc[:, go], hT[:, ab], hT[:, ab],
                                         start=True, stop=True)
                    nc.scalar.activation(expv[:], pssc[:], AF.Exp)
                    nc.gpsimd.tensor_mul(mexp[:], expv[:], blkmask[:])
                    nc.vector.tensor_reduce(
                        den[:], mexp[:].rearrange("p (g q) -> p g q", g=NGRP),
                        mybir.AxisListType.X, OP.add)
                    nc.vector.reciprocal(rden[:], den[:])
                    if stage < 4:
                        continue
                    rdenb = bass.AP(tensor=rden.tensor, offset=rden.offset,
                                    ap=[rden.ap[0], [rden.ap[1][0], NGRP],
                                        [0, 128]])
                    nc.vector.tensor_tensor(attn[:].rearrange(
                        "p (g q) -> p g q", g=NGRP), mexp[:].rearrange(
                        "p (g q) -> p g q", g=NGRP), rdenb, OP.mult)
                    for g in range(NGRP):
                        go = ts(g, 128)
                        ab = ds(i * NB + g * 128, 128)
                        nc.tensor.transpose(psth[:, go], hT[:, ab], ident[:])
                        nc.tensor.transpose(psta[:, go], attn[:, go], ident[:])
                    nc.vector.tensor_copy(hrow[:], psth[:])
                    nc.vector.tensor_copy(attnT[:], psta[:])
                    for g in range(NGRP):
                        go = ts(g, 128)
                        nc.tensor.matmul(psctx[:, go], hrow[:, go], attnT[:, go],
                                         start=True, stop=True)
                    nc.scalar.copy(ctxT[:], psctx[:])
                    if stage < 5:
                        continue
                    psfc = psum.tile([128, NB], F32, tag="pfc")
                    nc.tensor.matmul(psfc[:], wat1[:], hT[:, sl],
                                     start=True, stop=False)
                    nc.tensor.matmul(psfc[:], wat2[:], ctxT[:],
                                     start=False, stop=True)
                    nc.scalar.activation(hT[:, sl], psfc[:], AF.Identity,
                                         bias=battn[:, 0:1])
                    if stage < 6:
                        continue
                    # mu/std: [128 peds, 4] per group
                    psms = psum.tile([128, NGRP * 4], F32, tag="pfc")
                    for g in range(NGRP):
                        ab = ds(i * NB + g * 128, 128)
                        nc.tensor.matmul(psms[:, ts(g, 4)], hT[:, ab], wms[:],
                                         start=True, stop=False)
                        nc.tensor.matmul(psms[:, ts(g, 4)], onesb[:, 0:128],
                                         bms[:], start=False, stop=True)
                    nc.vector.tensor_copy(msbuf[:, ds(16 * i, 16)], psms[:])
                if stage < 6:
                    continue
                # std cols -> exp(0.5*x) in place, then one DMA for the timestep
                nc.scalar.activation(
                    msbuf[:].rearrange("p (i c) -> p i c", c=4)[:, :, 2:4],
                    msbuf[:].rearrange("p (i c) -> p i c", c=4)[:, :, 2:4],
                    AF.Exp, scale=0.5)
                nc.sync.dma_start(
                    d_out[t].rearrange("g c p -> p (g c)"), msbuf[:])

    nc.compile()
    return nc


def _host_pack(inputs, bl=BL, t_steps=TT, ncores=NCORES):
    """Slice + lay out the full inputs into per-core in_maps (layout prep only)."""
    f32 = np.float32
    enc = np.asarray(inputs["enc_h_feat"], f32)
    zz = np.asarray(inputs["z"], f32)
    ls = np.asarray(inputs["last_state"], f32)
    fut = np.asarray(inputs["fut_state"], f32)
    W_dec = np.asarray(inputs["W_dec"], f32); b_dec = np.asarray(inputs["b_dec"], f32)
    W_vel = np.asarray(inputs["W_vel"], f32); b_vel = np.asarray(inputs["b_vel"], f32)
    W_ih = np.asarray(inputs["W_ih"], f32); b_ih = np.asarray(inputs["b_ih"], f32)
    W_hh = np.asarray(inputs["W_hh"], f32); b_hh = np.asarray(inputs["b_hh"], f32)
    W_attn = np.asarray(inputs["W_attn"], f32); b_attn = np.asarray(inputs["b_attn"], f32)
    W_mu = np.asarray(inputs["W_mu"], f32); b_mu = np.asarray(inputs["b_mu"], f32)
    W_std = np.asarray(inputs["W_std"], f32); b_std = np.asarray(inputs["b_std"], f32)

    zxT = np.ascontiguousarray(np.concatenate([enc, zz], axis=1).T)      # [288, B]
    lsT = np.ascontiguousarray(ls.T)                                     # [6, B]
    futT = np.ascontiguousarray(fut.transpose(0, 2, 1)).reshape(t_steps * NP, -1)
    futT = futT.astype(np.float16)

    whhT = np.ascontiguousarray(W_hh.T).astype(np.float32)
    whhT[:, 256:384] *= 0.5
    whhT = whhT.astype(np.float16)                                       # [128, 384]
    bhh05 = (0.5 * b_hh[256:384]).reshape(1, 128).astype(f32)
    wiaT = np.ascontiguousarray(W_ih[:, ZX:].T).astype(np.float16)           # [2, 384]
    wihxT = np.ascontiguousarray(W_ih[:, :ZX].T)                         # [288, 384]
    wdecT = np.ascontiguousarray(W_dec.T)                                # [288, 128]
    wvelT = np.ascontiguousarray(W_vel.T)                                # [6, 2]
    wattnT = np.ascontiguousarray(W_attn.T).astype(np.float16)               # [256, 128]
    wms = np.ascontiguousarray(
        np.concatenate([W_mu, W_std], axis=0).T).astype(np.float16)          # [128, 4]
    bms = np.concatenate([b_mu, b_std]).reshape(1, 4).astype(np.float16)
    bvel = b_vel.reshape(1, NP).astype(f32)
    biasgx = np.stack([
        b_ih[0:128] + b_hh[0:128],
        b_ih[128:256] + b_hh[128:256],
        b_ih[256:384],
    ], axis=1).astype(f32)                                               # [128, 3]
    bhhn = b_hh[256:384].reshape(H, 1).astype(f32)
    bhhnh = (0.5 * b_hh[256:384]).reshape(H, 1).astype(f32)
    battn2 = b_attn.reshape(H, 1).astype(f32)
    bdec2 = b_dec.reshape(H, 1).astype(f32)
    ident = np.eye(128, dtype=np.float16)
    blk1 = np.kron(np.eye(128 // PED, dtype=f32), np.ones((PED, PED), f32))
    blk = np.tile(blk1, (1, NB // 128)).astype(BF16NP)                   # [128, NB]
    onesb = np.ones((1, NB), np.float16)
    onesf = np.ones((1, NB), f32)

    shared = dict(whhT=whhT, wiaT=wiaT, wihxT=wihxT, wdecT=wdecT, wvelT=wvelT,
                  wat2f=np.ascontiguousarray(W_attn.T[128:256]).astype(f32),
                  bhh05=bhh05,
                  wattnT=wattnT, wms=wms, bms=bms, bvel=bvel, biasgx=biasgx,
                  bhhn=bhhn, bhhnh=bhhnh, battn=battn2, bdec=bdec2, ident=ident,
                  blkmask=blk, onesb=onesb, onesf=onesf)
    in_maps = []
    for c in range(ncores):
        sl = slice(c * bl, (c + 1) * bl)
        m = dict(shared)
        m["zxT"] = np.ascontiguousarray(zxT[:, sl])
        m["lsT"] = np.ascontiguousarray(lsT[:, sl])
        m["futT"] = np.ascontiguousarray(futT[:, sl])
        in_maps.append(m)
    return in_maps


def _assemble(results, bl=BL, t_steps=TT):
    """results: per-core dicts with outT [T, bl//128, 4, 128] -> (mus, stds)."""
    outs = np.concatenate([r["outT"] for r in results], axis=1)  # [T, B/128, 4, 128]
    o = outs.transpose(0, 1, 3, 2).reshape(t_steps, -1, 4)       # [T, B, 4]
    mus = np.ascontiguousarray(o[:, :, 0:2])
    stds = np.ascontiguousarray(o[:, :, 2:4])
    return mus, stds


_NC_CACHE = {}


def run_kernel(inputs, trace=False, **kw):
    from concourse.bass_utils import run_bass_kernel_spmd
    key = "full"
    if key not in _NC_CACHE:
        _NC_CACHE[key] = build_module()
    nc = _NC_CACHE[key]
    in_maps = _host_pack(inputs)
    res = run_bass_kernel_spmd(nc, in_maps, core_ids=list(range(NCORES)),
                               trace=trace, **kw)
    mus, stds = _assemble(res.results)
    return mus, stds, res


def kernel(**inputs):
    mus, stds, _ = run_kernel(inputs)
    return mus, stds


if __name__ == "__main__":
    pass

